# revision 1
# baseline (speedup 1.0000x reference)
"""Distributed multi-head attention kernel for 8 Trainium2 NeuronCores.

Problem: y = softmax((x Wq^T)(x Wk^T)^T / sqrt(D)) (x Wv^T) Wo^T + bo
with B=4, T=2048, C=1280, H=20, D=64, float32 I/O.

Sharding (sequence parallel, rank independent):
  Each core owns a T/8 token slice of all 4 batches (1024 tokens).
  It computes Q/K/V projections for its tokens, AllGathers K^T and V
  (bf16, chunked per batch pair so comm overlaps compute), runs full
  attention for its queries over the gathered keys/values of the
  matching batch, and applies the output projection for its tokens.
  The host reassembles the T axis.

Attention runs in a transposed "S_T[k, q]" layout so the softmax
denominator falls out of the same matmul that computes P@V: V is
stored padded per head as [.., 65] with a constant-1 column, so psum
row 64 of the P@V accumulation is sum_k P. This avoids partition-axis
reductions entirely. exp() runs on ScalarE directly out of PSUM in
[128, 4*TS] chunks; P@V is interleaved per chunk so the PE keeps
working while ScalarE drains. Q-projection for batches 2-3 is emitted
after batch-0 attention as PE gap filler.

Compute dtype is bf16 (fp32 matmul is 4x slower on the PE array);
accumulation is fp32 in PSUM. I/O stays fp32.
"""

import os
import sys
import types

import numpy as np
import ml_dtypes

import concourse.bass as bass
import concourse.mybir as mybir
import concourse.tile as tile
from concourse import bacc
from concourse.bass_utils import run_bass_kernel_spmd

N_CORES = 8
C = 1280
H = 20
D = 64
B = 4
CT = C // 128  # 10 c-tiles
HP5 = H * 65   # padded V row width (64 dims + ones column per head)
BF = mybir.dt.bfloat16
F32 = mybir.dt.float32
SCALE = 1.0 / (D ** 0.5)

LAST_EXEC_TIME_NS = None
_BUILD_CACHE = {}


def _install_ntff_hook():
    """The trimmed antenv package lacks axon_hooks; register the NTFF
    profile hook by hand so trace=True can time the NEFF on silicon.
    Safe no-op if anything is missing."""
    if "antenv.axon_hooks" in sys.modules:
        return
    try:
        from trn_agent_boot.trn_boot import _ntff_profile_via_ctypes

        hook = _ntff_profile_via_ctypes("/opt/axon/libaxon_pjrt.so")
        mod = types.ModuleType("antenv.axon_hooks")
        mod.get_axon_ntff_profile_hook = lambda: hook
        mod.set_axon_ntff_profile_hook = lambda h: None
        sys.modules["antenv.axon_hooks"] = mod
        import antenv

        antenv.axon_hooks = mod
    except Exception:
        pass


def _chunks(total, step):
    out = []
    o = 0
    while o < total:
        out.append((o, min(step, total - o)))
        o += step
    return out


def build(T):
    """Build the SPMD Bass graph for full (unsharded) sequence length T."""
    TS = T // N_CORES          # tokens per batch per core
    TOK = B * TS               # tokens per core
    KT = T // 128              # 128-wide key tiles per batch
    assert TS % 128 == 0, "key tiles must not cross rank chunks"
    assert TS <= 512, "q tile must fit one matmul moving operand"
    assert KT % 4 == 0
    C4 = KT // 4               # exp chunks (4 key tiles each) per head
    TH = 2 * TS                # tokens per batch pair per core
    RG = [list(range(N_CORES))]

    nc = bacc.Bacc("TRN2", target_bir_lowering=False, debug=False,
                   num_devices=N_CORES)

    xT = nc.dram_tensor("xT", [C, TOK], BF, kind="ExternalInput").ap()
    wqT = nc.dram_tensor("wqT", [C, C], BF, kind="ExternalInput").ap()
    wkT = nc.dram_tensor("wkT", [C, C], BF, kind="ExternalInput").ap()
    wvT = nc.dram_tensor("wvT", [C, C], BF, kind="ExternalInput").ap()
    woT = nc.dram_tensor("woT", [C, C], BF, kind="ExternalInput").ap()
    bo_d = nc.dram_tensor("bo", [C, 1], F32, kind="ExternalInput").ap()
    out = nc.dram_tensor("out", [C, TOK], F32, kind="ExternalOutput").ap()

    with tile.TileContext(nc) as tc:
        with tc.tile_pool(name="dram", bufs=1, space="DRAM") as dram:
            # per batch-pair bounce + gathered buffers (K and V fused into
            # one AllGather payload per half)
            SZK = C * TH
            SZV = TH * HP5
            SZ = SZK + SZV
            kv_bn = [dram.tile([SZ], BF, name=f"kv_bn{i}") for i in range(2)]
            kv_all = [dram.tile([N_CORES * SZ], BF, addr_space="Shared",
                                name=f"kv_all{i}") for i in range(2)]
            k_bn = [t[0:SZK].rearrange("(r t) -> r t", t=TH) for t in kv_bn]
            v_bn = [t[SZK:SZ].rearrange("(p c) -> p c", c=HP5) for t in kv_bn]
            scr_d = dram.tile([128, 512], F32, name="scr_d")

            with tc.tile_pool(name="persist", bufs=1) as persist:
                qT_sb = persist.tile([128, CT, TOK], BF)
                attn_sb = persist.tile([128, CT, TOK], BF)
                ones_sb = persist.tile([128, 64], BF)
                nc.vector.memset(ones_sb[:], 1.0)
                wo_sb = persist.tile([128, CT, C], BF)
                bo_sb = persist.tile([128, CT, 1], F32)

                def proj_T(psum, w_sb, dram_dst, sbuf_dst, t_lo, t_hi, pool,
                           dst_ofs=0, psum_tag="mm", psum_bufs=6):
                    # out[o, t] = sum_i W^T[i, o] x^T[i, t]
                    for ot in range(CT):
                        for t0, tsz in _chunks(t_hi - t_lo, 512):
                            t0 += t_lo
                            ps = psum.tile([128, 512], F32, tag=psum_tag,
                                           bufs=psum_bufs, name="ps_proj")
                            for i in range(CT):
                                nc.tensor.matmul(
                                    ps[:, :tsz],
                                    w_sb[:, i, ot * 128:(ot + 1) * 128],
                                    xT_sb[:, i, t0:t0 + tsz],
                                    start=(i == 0), stop=(i == CT - 1))
                            if sbuf_dst is not None:
                                nc.vector.tensor_copy(
                                    sbuf_dst[:, ot, t0:t0 + tsz],
                                    ps[:, :tsz])
                            else:
                                st = pool.tile([128, 512], BF, tag="st",
                                               bufs=4, name="st_proj")
                                nc.vector.tensor_copy(st[:, :tsz],
                                                      ps[:, :tsz])
                                nc.sync.dma_start(
                                    dram_dst[ot * 128:(ot + 1) * 128,
                                             t0 - dst_ofs:
                                             t0 - dst_ofs + tsz],
                                    st[:, :tsz])

                def proj_V(psum, half, pool):
                    # V (token-major, per-head 65-padded with a ones col)
                    for ttl in range(TH // 128):
                        tt = half * (TH // 128) + ttl
                        stv = pool.tile([128, H, 65], BF, tag="stv", bufs=2,
                                        name="stv")
                        nc.vector.memset(stv[:, :, 64:65], 1.0)
                        for o0, osz in _chunks(C, 512):
                            ps = psum.tile([128, 512], F32, tag="mm",
                                           bufs=6, name="ps_v")
                            for i in range(CT):
                                nc.tensor.matmul(
                                    ps[:, :osz],
                                    xT_sb[:, i, tt * 128:(tt + 1) * 128],
                                    wv_sb[:, i, o0:o0 + osz],
                                    start=(i == 0), stop=(i == CT - 1))
                            h0, nh = o0 // 64, osz // 64
                            nc.vector.tensor_copy(
                                stv[:, h0:h0 + nh, 0:64],
                                ps[:, :osz].rearrange("p (h c) -> p h c",
                                                      c=64))
                        nc.sync.dma_start(
                            v_bn[half][ttl * 128:(ttl + 1) * 128, :],
                            stv[:].rearrange("p h c -> p (h c)"))

                # ---------------- Phase 1: K/V projections + AGs ----------
                with tc.tile_pool(name="p1", bufs=1) as p1, \
                     tc.tile_pool(name="psum1", bufs=1, space="PSUM") as psum1:
                    xT_sb = p1.tile([128, CT, TOK], BF)
                    nc.sync.dma_start(
                        xT_sb[:], xT.rearrange("(n p) t -> p n t", p=128))
                    wq_sb = p1.tile([128, CT, C], BF)
                    nc.sync.dma_start(
                        wq_sb[:], wqT.rearrange("(n p) o -> p n o", p=128))
                    wk_sb = p1.tile([128, CT, C], BF)
                    nc.sync.dma_start(
                        wk_sb[:], wkT.rearrange("(n p) o -> p n o", p=128))
                    wv_sb = p1.tile([128, CT, C], BF)
                    nc.sync.dma_start(
                        wv_sb[:], wvT.rearrange("(n p) o -> p n o", p=128))

                    for half in range(2):
                        proj_T(psum1, wk_sb, k_bn[half], None,
                               half * TH, (half + 1) * TH, p1,
                               dst_ofs=half * TH)
                        proj_V(psum1, half, p1)
                        nc.gpsimd.collective_compute(
                            "AllGather", mybir.AluOpType.bypass,
                            replica_groups=RG,
                            ins=[kv_bn[half][:].opt()],
                            outs=[kv_all[half][:].opt()])

                    # all of Q^T (overlaps the AllGathers)
                    proj_T(psum1, wq_sb, None, qT_sb, 0, TOK, p1)

                    nc.sync.dma_start(
                        wo_sb[:], woT.rearrange("(n p) o -> p n o", p=128))
                    nc.sync.dma_start(
                        bo_sb[:], bo_d.rearrange("(n p) o -> p n o", p=128))

                # ------------- Phase 2+3: attention + out-proj -------------
                with tc.tile_pool(name="p2", bufs=1) as p2, \
                     tc.tile_pool(name="psum2", bufs=1, space="PSUM") as psum2:
                    JR = TS // 128  # key tiles per rank chunk
                    # dedicated scratch bank: idempotent matmuls that keep
                    # the PE activity monitor from re-throttling the clock
                    # while ScalarE drains exp chunks
                    scr = psum2.tile([64, TS], F32, tag="scr", bufs=1,
                                     name="scr")

                    for b in range(B):
                        half, bb = b // 2, b % 2
                        kv_s = kv_all[half][:].rearrange(
                            "(s x) -> s x", s=N_CORES)
                        k_all_v = kv_s[:, 0:SZK].rearrange(
                            "s (r t) -> r s t", t=TH)
                        v_all_v = kv_s[:, SZK:SZ].rearrange(
                            "s (j p c) -> s j p c", p=128, c=HP5)

                        # V for this batch, two halves of key tiles
                        vbs = []
                        for kh in range(2):
                            vb = p2.tile([128, KT // 2, HP5], BF, tag="vb",
                                         bufs=2, name=f"vb{kh}")
                            for s0 in range(N_CORES // 2):
                                s = kh * (N_CORES // 2) + s0
                                nc.sync.dma_start(
                                    vb[:, s0 * JR:(s0 + 1) * JR, :],
                                    v_all_v[s, bb * JR:(bb + 1) * JR, :, :]
                                    .rearrange("j p c -> p j c"))
                            vbs.append(vb)

                        for hp in range(CT):
                            kp = p2.tile([128, N_CORES, TS], BF, tag="kp",
                                         bufs=2, name="kp")
                            nc.sync.dma_start(
                                kp[:],
                                k_all_v[hp * 128:(hp + 1) * 128, :,
                                        bb * TS:(bb + 1) * TS])
                            kp_f = kp[:].rearrange("p s t -> p (s t)")

                            P0 = p2.tile([128, KT * TS], BF, tag="P",
                                         bufs=2, name="P0")
                            P1 = p2.tile([128, KT * TS], BF, tag="P",
                                         bufs=2, name="P1")
                            pav0 = psum2.tile([65, TS], F32, tag="pav",
                                              bufs=2, name="pav0")
                            pav1 = psum2.tile([65, TS], F32, tag="pav",
                                              bufs=2, name="pav1")
                            pavs = (pav0, pav1)
                            for c4 in range(C4):
                                # separate tags -> stable separate banks per
                                # head so paired row-group matmuls can run
                                # concurrently on the PE
                                psA = psum2.tile([128, 4 * TS], F32,
                                                 tag="ps_a", bufs=1,
                                                 name="psA")
                                psB = psum2.tile([128, 4 * TS], F32,
                                                 tag="ps_b", bufs=1,
                                                 name="psB")
                                for j in range(4):
                                    kt = c4 * 4 + j
                                    for h, ps in ((0, psA), (1, psB)):
                                        nc.tensor.matmul(
                                            ps[:, j * TS:(j + 1) * TS],
                                            kp_f[h * 64:(h + 1) * 64,
                                                 kt * 128:(kt + 1) * 128],
                                            qT_sb[h * 64:(h + 1) * 64, hp,
                                                  b * TS:(b + 1) * TS],
                                            start=True, stop=True,
                                            tile_position=(h * 64, 0))
                                for ps, P in ((psA, P0), (psB, P1)):
                                    nc.scalar.activation(
                                        P[:, c4 * 4 * TS:(c4 + 1) * 4 * TS],
                                        ps[:],
                                        mybir.ActivationFunctionType.Exp,
                                        scale=SCALE)
                                for _ in range(2):
                                    nc.tensor.matmul(
                                        scr[:], ones_sb[:, 0:64],
                                        qT_sb[:, hp, b * TS:(b + 1) * TS],
                                        start=True, stop=True)
                                for h, P in ((0, P0), (1, P1)):
                                    hg = 2 * hp + h
                                    for j in range(4):
                                        kt = c4 * 4 + j
                                        vb = vbs[0] if kt < KT // 2 else vbs[1]
                                        ktl = kt % (KT // 2)
                                        nc.tensor.matmul(
                                            pavs[h][:],
                                            vb[:, ktl,
                                               hg * 65:(hg + 1) * 65],
                                            P[:, kt * TS:(kt + 1) * TS],
                                            start=(kt == 0),
                                            stop=(kt == KT - 1))

                            for h in (0, 1):
                                recip = p2.tile([128, TS], BF, tag="recip",
                                                bufs=2, name="recip")
                                with nc.allow_low_precision(
                                        reason="softmax denom in bf16"):
                                    nc.vector.reciprocal(recip[64:65, :],
                                                         pavs[h][64:65, :])
                                pbc = psum2.tile([64, TS], F32, tag="pbc",
                                                 bufs=1, name="pbc")
                                nc.tensor.matmul(
                                    pbc[:], ones_sb[64:65, :],
                                    recip[64:65, :],
                                    start=True, stop=True)
                                bcast_sb = p2.tile([64, TS], F32,
                                                   tag="bcast", bufs=2,
                                                   name="bcast_sb")
                                nc.vector.tensor_copy(bcast_sb[:], pbc[:])
                                tmp = p2.tile([64, TS], BF, tag="tmp",
                                              bufs=3, name="tmp")
                                nc.vector.tensor_mul(tmp[:], pavs[h][0:64, :],
                                                     bcast_sb[:])
                                nc.sync.dma_start(
                                    attn_sb[h * 64:(h + 1) * 64, hp,
                                            b * TS:(b + 1) * TS],
                                    tmp[:])

                        # out projection for this batch's tokens
                        for co in range(CT):
                            psy = psum2.tile([128, TS], F32, tag="ps_a",
                                             bufs=1, name="psy")
                            for ct in range(CT):
                                nc.tensor.matmul(
                                    psy[:],
                                    wo_sb[:, ct, co * 128:(co + 1) * 128],
                                    attn_sb[:, ct, b * TS:(b + 1) * TS],
                                    start=(ct == 0), stop=(ct == CT - 1))
                            ysb = p2.tile([128, TS], F32, tag="y", bufs=3,
                                          name="ysb")
                            nc.vector.tensor_scalar_add(
                                ysb[:], psy[:], bo_sb[:, co, :])
                            nc.sync.dma_start(
                                out[co * 128:(co + 1) * 128,
                                    b * TS:(b + 1) * TS],
                                ysb[:])

                    scr_sb = p2.tile([64, TS], F32, name="scr_sb")
                    nc.vector.tensor_copy(scr_sb[:], scr[:])
                    nc.sync.dma_start(scr_d[0:64, 0:TS], scr_sb[:])

    nc.compile()
    return nc


def _prep_inputs(hidden_states, Wq, Wk, Wv, Wo, bo):
    T = hidden_states.shape[1]
    TS = T // N_CORES
    TOK = B * TS
    bf = ml_dtypes.bfloat16
    wqT = np.ascontiguousarray(np.asarray(Wq, np.float32).T).astype(bf)
    wkT = np.ascontiguousarray(np.asarray(Wk, np.float32).T).astype(bf)
    wvT = np.ascontiguousarray(np.asarray(Wv, np.float32).T).astype(bf)
    woT = np.ascontiguousarray(np.asarray(Wo, np.float32).T).astype(bf)
    bo_c = np.asarray(bo, np.float32).reshape(C, 1)
    x = np.asarray(hidden_states, np.float32)
    in_maps = []
    for r in range(N_CORES):
        xr = x[:, r * TS:(r + 1) * TS, :].reshape(TOK, C)
        xTr = np.ascontiguousarray(xr.T).astype(bf)
        in_maps.append({
            "xT": xTr, "wqT": wqT, "wkT": wkT, "wvT": wvT, "woT": woT,
            "bo": bo_c,
        })
    return in_maps


def kernel(hidden_states, Wq, Wk, Wv, Wo, bo):
    global LAST_EXEC_TIME_NS
    _install_ntff_hook()
    Bx, T, Cx = hidden_states.shape
    assert (Bx, Cx) == (B, C)
    TS = T // N_CORES
    if T not in _BUILD_CACHE:
        _BUILD_CACHE[T] = build(T)
    nc = _BUILD_CACHE[T]
    in_maps = _prep_inputs(hidden_states, Wq, Wk, Wv, Wo, bo)
    res = run_bass_kernel_spmd(nc, in_maps, core_ids=list(range(N_CORES)))
    LAST_EXEC_TIME_NS = res.exec_time_ns
    outf = np.empty((B, T, C), np.float32)
    for r in range(N_CORES):
        yT = res.results[r]["out"]          # [C, TOK]
        yr = yT.T.reshape(B, TS, C)
        outf[:, r * TS:(r + 1) * TS, :] = yr
    return outf



# revision 2
# speedup vs baseline: 1.4093x; 1.4093x over previous
"""Distributed multi-head attention kernel for 8 Trainium2 NeuronCores.

Problem: y = softmax((x Wq^T)(x Wk^T)^T / sqrt(D)) (x Wv^T) Wo^T + bo
with B=4, T=2048, C=1280, H=20, D=64, float32 I/O.

Sharding (sequence parallel, rank independent):
  Each core owns a T/8 token slice of all 4 batches (1024 tokens).
  It computes Q/K/V projections for its tokens, AllGathers K^T and V
  per batch (4 pipelined AllGathers so comm overlaps compute), runs
  full attention for its queries over the gathered keys/values, and
  applies the output projection for its tokens. The host reassembles
  the T axis.

The device is power/utilization-throttled under sustained PE load, so
the design minimizes PE busy-cycles:
  - Scores run in transposed S_T[k, q] layout, two heads row-packed
    (tile_position (0,0)/(64,0)) at full array width.
  - P@V runs column-tiled: both heads of a pair concurrently at M=64
    (tile_position (0,0)/(0,64)) - 2x over the padded-V M=65 scheme.
  - Softmax denominators come from 4-way column-tiled ones-matmuls
    (M=32 strips, two kt streams x two heads per slot).
  - Normalization: one expander matmul broadcasts summed strips to all
    128 rows, reciprocal_approx_fast + one tensor_mul per head pair.
  - No filler matmuls; O-projection is interleaved into the next
    batch's attention loop to keep ScalarE (exp) saturated.

Compute dtype is bf16 (fp32 matmul is 4x slower on the PE array);
accumulation is fp32 in PSUM. I/O stays fp32.
"""

import os
import sys
import types

import numpy as np
import ml_dtypes

import concourse.bass as bass
import concourse.mybir as mybir
import concourse.tile as tile
from concourse import bacc
from concourse.bass_utils import run_bass_kernel_spmd

N_CORES = 8
C = 1280
H = 20
D = 64
B = 4
CT = C // 128  # 10 c-tiles
BF = mybir.dt.bfloat16
F32 = mybir.dt.float32
SCALE = 1.0 / (D ** 0.5)

LAST_EXEC_TIME_NS = None
_BUILD_CACHE = {}


def _install_ntff_hook():
    """The trimmed antenv package lacks axon_hooks; register the NTFF
    profile hook by hand so trace=True can time the NEFF on silicon.
    Safe no-op if anything is missing."""
    if "antenv.axon_hooks" in sys.modules:
        return
    try:
        from trn_agent_boot.trn_boot import _ntff_profile_via_ctypes

        hook = _ntff_profile_via_ctypes("/opt/axon/libaxon_pjrt.so")
        mod = types.ModuleType("antenv.axon_hooks")
        mod.get_axon_ntff_profile_hook = lambda: hook
        mod.set_axon_ntff_profile_hook = lambda h: None
        sys.modules["antenv.axon_hooks"] = mod
        import antenv

        antenv.axon_hooks = mod
    except Exception:
        pass


def _chunks(total, step):
    out = []
    o = 0
    while o < total:
        out.append((o, min(step, total - o)))
        o += step
    return out


def build(T):
    """Build the SPMD Bass graph for full (unsharded) sequence length T."""
    TS = T // N_CORES          # tokens per batch per core (256)
    TOK = B * TS               # tokens per core (1024)
    KT = T // 128              # 128-wide key tiles per batch (16)
    JR = TS // 128             # key tiles per rank per batch (2)
    assert TS % 128 == 0 and KT % 4 == 0
    C4 = KT // 4               # exp chunks (4 key tiles each) per head
    RG = [list(range(N_CORES))]
    SZK = C * TS               # K^T payload elems per batch
    SZV = TS * C               # V payload elems per batch
    SZ = SZK + SZV

    nc = bacc.Bacc("TRN2", target_bir_lowering=False, debug=False,
                   num_devices=N_CORES)

    xT = nc.dram_tensor("xT", [C, TOK], BF, kind="ExternalInput").ap()
    wqT = nc.dram_tensor("wqT", [C, C], BF, kind="ExternalInput").ap()
    wkT = nc.dram_tensor("wkT", [C, C], BF, kind="ExternalInput").ap()
    wvT = nc.dram_tensor("wvT", [C, C], BF, kind="ExternalInput").ap()
    woT = nc.dram_tensor("woT", [C, C], BF, kind="ExternalInput").ap()
    bo_d = nc.dram_tensor("bo", [C, 1], F32, kind="ExternalInput").ap()
    # expander: bc_den[p, q] = sum of the two den strips of head(p)
    esel_d = nc.dram_tensor("esel", [128, 128], BF, kind="ExternalInput").ap()
    out = nc.dram_tensor("out", [C, TOK], F32, kind="ExternalOutput").ap()

    with tile.TileContext(nc) as tc:
        with tc.tile_pool(name="dram", bufs=1, space="DRAM") as dram:
            kv_bn = [dram.tile([SZ], BF, name=f"kv_bn{b}") for b in range(B)]
            kv_all = [dram.tile([N_CORES * SZ], BF, addr_space="Shared",
                                name=f"kv_all{b}") for b in range(B)]

            with tc.tile_pool(name="persist", bufs=1) as persist:
                qT_sb = persist.tile([128, CT, TOK], BF)
                attn_sb = persist.tile([128, CT, TOK], BF)
                wo_sb = persist.tile([128, CT, C], BF)
                bo_sb = persist.tile([128, CT, 1], F32)
                ones32 = persist.tile([128, 32], BF)
                nc.vector.memset(ones32[:], 1.0)
                esel_sb = persist.tile([128, 128], BF)
                nc.sync.dma_start(esel_sb[:], esel_d)

                # ---------------- Phase 1: projections + AGs ----------
                with tc.tile_pool(name="p1", bufs=1) as p1, \
                     tc.tile_pool(name="psum1", bufs=1, space="PSUM") as psum1:
                    xT_sb = p1.tile([128, CT, TOK], BF)
                    nc.sync.dma_start(
                        xT_sb[:], xT.rearrange("(n p) t -> p n t", p=128))
                    wk_sb = p1.tile([128, CT, C], BF)
                    nc.sync.dma_start(
                        wk_sb[:], wkT.rearrange("(n p) o -> p n o", p=128))
                    wv_sb = p1.tile([128, CT, C], BF)
                    nc.sync.dma_start(
                        wv_sb[:], wvT.rearrange("(n p) o -> p n o", p=128))
                    wq_sb = p1.tile([128, CT, C], BF)
                    nc.sync.dma_start(
                        wq_sb[:], wqT.rearrange("(n p) o -> p n o", p=128))

                    for half in range(2):
                        t_lo = half * 2 * TS
                        # K^T projection for this half's 512 tokens
                        for ot in range(CT):
                            ps = psum1.tile([128, 2 * TS], F32, tag="mm",
                                            bufs=4, name="ps_k")
                            for i in range(CT):
                                nc.tensor.matmul(
                                    ps[:],
                                    wk_sb[:, i, ot * 128:(ot + 1) * 128],
                                    xT_sb[:, i, t_lo:t_lo + 2 * TS],
                                    start=(i == 0), stop=(i == CT - 1))
                            st = p1.tile([128, 2 * TS], BF, tag="st",
                                         bufs=4, name="st_k")
                            nc.vector.tensor_copy(st[:], ps[:])
                            for bb in range(2):
                                b = 2 * half + bb
                                kview = kv_bn[b][0:SZK].rearrange(
                                    "(r t) -> r t", t=TS)
                                nc.sync.dma_start(
                                    kview[ot * 128:(ot + 1) * 128, :],
                                    st[:, bb * TS:(bb + 1) * TS])
                        # V projection (token-major) + AG per batch
                        for bb in range(2):
                            b = 2 * half + bb
                            vview = kv_bn[b][SZK:SZ].rearrange(
                                "(p c) -> p c", c=C)
                            for ttl in range(JR):
                                tt = b * JR + ttl
                                stv = p1.tile([128, C], BF, tag="stv",
                                              bufs=3, name="stv")
                                for o0, osz in _chunks(C, 512):
                                    ps = psum1.tile([128, 512], F32, tag="mm",
                                                    bufs=4, name="ps_v")
                                    for i in range(CT):
                                        nc.tensor.matmul(
                                            ps[:, :osz],
                                            xT_sb[:, i,
                                                  tt * 128:(tt + 1) * 128],
                                            wv_sb[:, i, o0:o0 + osz],
                                            start=(i == 0),
                                            stop=(i == CT - 1))
                                    nc.vector.tensor_copy(
                                        stv[:, o0:o0 + osz], ps[:, :osz])
                                nc.sync.dma_start(
                                    vview[ttl * 128:(ttl + 1) * 128, :],
                                    stv[:])
                            nc.gpsimd.collective_compute(
                                "AllGather", mybir.AluOpType.bypass,
                                replica_groups=RG,
                                ins=[kv_bn[b][:].opt()],
                                outs=[kv_all[b][:].opt()])

                    # all of Q^T (overlaps the AllGathers)
                    for ot in range(CT):
                        for t0, tsz in _chunks(TOK, 512):
                            ps = psum1.tile([128, 512], F32, tag="mm",
                                            bufs=4, name="ps_q")
                            for i in range(CT):
                                nc.tensor.matmul(
                                    ps[:, :tsz],
                                    wq_sb[:, i, ot * 128:(ot + 1) * 128],
                                    xT_sb[:, i, t0:t0 + tsz],
                                    start=(i == 0), stop=(i == CT - 1))
                            nc.vector.tensor_copy(
                                qT_sb[:, ot, t0:t0 + tsz], ps[:, :tsz])

                    nc.sync.dma_start(
                        wo_sb[:], woT.rearrange("(n p) o -> p n o", p=128))
                    nc.sync.dma_start(
                        bo_sb[:], bo_d.rearrange("(n p) o -> p n o", p=128))

                # ------------- Phase 2: attention + out-proj -------------
                with tc.tile_pool(name="p2", bufs=1) as p2, \
                     tc.tile_pool(name="psum2", bufs=1, space="PSUM") as psum2:

                    def load_vbs(b):
                        kv_s = kv_all[b][:].rearrange("(s x) -> s x",
                                                      s=N_CORES)
                        v_all_v = kv_s[:, SZK:SZ].rearrange(
                            "s (j p c) -> s j p c", p=128, c=C)
                        vbs = []
                        for kh in range(2):
                            vb = p2.tile([128, KT // 2, C], BF, tag="vb",
                                         bufs=4, name=f"vb{kh}")
                            for s0 in range(N_CORES // 2):
                                s = kh * (N_CORES // 2) + s0
                                nc.sync.dma_start(
                                    vb[:, s0 * JR:(s0 + 1) * JR, :],
                                    v_all_v[s].rearrange("j p c -> p j c"))
                            vbs.append(vb)
                        return vbs

                    def emit_oproj(b, co):
                        psy = psum2.tile([128, TS], F32, tag="misc", bufs=1,
                                         name="psy")
                        for ct in range(CT):
                            nc.tensor.matmul(
                                psy[:],
                                wo_sb[:, ct, co * 128:(co + 1) * 128],
                                attn_sb[:, ct, b * TS:(b + 1) * TS],
                                start=(ct == 0), stop=(ct == CT - 1))
                        ysb = p2.tile([128, TS], F32, tag="y", bufs=3,
                                      name="ysb")
                        nc.vector.tensor_scalar_add(
                            ysb[:], psy[:], bo_sb[:, co, :])
                        nc.sync.dma_start(
                            out[co * 128:(co + 1) * 128,
                                b * TS:(b + 1) * TS],
                            ysb[:])

                    vbs_cur = load_vbs(0)
                    for b in range(B):
                        kv_s = kv_all[b][:].rearrange("(s x) -> s x",
                                                      s=N_CORES)
                        k_all_v = kv_s[:, 0:SZK].rearrange(
                            "s (r t) -> r s t", t=TS)
                        btok = slice(b * TS, (b + 1) * TS)

                        for hp in range(CT):
                            kp = p2.tile([128, N_CORES, TS], BF, tag="kp",
                                         bufs=2, name="kp")
                            nc.sync.dma_start(
                                kp[:], k_all_v[hp * 128:(hp + 1) * 128, :, :])
                            kp_f = kp[:].rearrange("p s t -> p (s t)")

                            P0 = p2.tile([128, KT * TS], BF, tag="P0",
                                         bufs=2, name="P0")
                            P1 = p2.tile([128, KT * TS], BF, tag="P1",
                                         bufs=2, name="P1")
                            pav = psum2.tile([128, TS], F32, tag="pav",
                                             bufs=2, name="pav")
                            den = psum2.tile([128, TS], F32, tag="den",
                                             bufs=1, name="den")
                            he, ho = 2 * hp, 2 * hp + 1

                            for c4 in range(C4):
                                spA = psum2.tile([128, 4 * TS], F32,
                                                 tag="spA", bufs=1,
                                                 name="spA")
                                spB = psum2.tile([128, 4 * TS], F32,
                                                 tag="spB", bufs=1,
                                                 name="spB")
                                for j in range(4):
                                    kt = c4 * 4 + j
                                    nc.tensor.matmul(
                                        spA[:, j * TS:(j + 1) * TS],
                                        kp_f[0:64,
                                             kt * 128:(kt + 1) * 128],
                                        qT_sb[0:64, hp, btok],
                                        start=True, stop=True,
                                        tile_position=(0, 0))
                                    nc.tensor.matmul(
                                        spB[:, j * TS:(j + 1) * TS],
                                        kp_f[64:128,
                                             kt * 128:(kt + 1) * 128],
                                        qT_sb[64:128, hp, btok],
                                        start=True, stop=True,
                                        tile_position=(64, 0))
                                nc.scalar.activation(
                                    P0[:, c4 * 4 * TS:(c4 + 1) * 4 * TS],
                                    spA[:],
                                    mybir.ActivationFunctionType.Exp,
                                    scale=SCALE)
                                nc.scalar.activation(
                                    P1[:, c4 * 4 * TS:(c4 + 1) * 4 * TS],
                                    spB[:],
                                    mybir.ActivationFunctionType.Exp,
                                    scale=SCALE)
                                # P@V, both heads column-tiled concurrent
                                for j in range(4):
                                    kt = c4 * 4 + j
                                    vb = vbs_cur[kt // (KT // 2)]
                                    ktl = kt % (KT // 2)
                                    nc.tensor.matmul(
                                        pav[0:64, :],
                                        vb[:, ktl, he * 64:he * 64 + 64],
                                        P0[:, kt * TS:(kt + 1) * TS],
                                        start=(kt == 0), stop=(kt == KT - 1),
                                        tile_position=(0, 0),
                                        skip_group_check=True)
                                    nc.tensor.matmul(
                                        pav[64:128, :],
                                        vb[:, ktl, ho * 64:ho * 64 + 64],
                                        P1[:, kt * TS:(kt + 1) * TS],
                                        start=(kt == 0), stop=(kt == KT - 1),
                                        tile_position=(0, 64),
                                        skip_group_check=True)
                                # denominator strips, 4-way column-tiled:
                                # rows 0-31 h_even/kt-even, 32-63 h_even/
                                # kt-odd, 64-95 h_odd/kt-even, 96-127
                                # h_odd/kt-odd
                                for j2 in range(2):
                                    kte = c4 * 4 + 2 * j2
                                    kto = kte + 1
                                    st0 = (c4 == 0 and j2 == 0)
                                    sp1 = (c4 == C4 - 1 and j2 == 1)
                                    for pos, P, kt in ((0, P0, kte),
                                                       (32, P0, kto),
                                                       (64, P1, kte),
                                                       (96, P1, kto)):
                                        nc.tensor.matmul(
                                            den[pos:pos + 32, :],
                                            ones32[:],
                                            P[:, kt * TS:(kt + 1) * TS],
                                            start=st0, stop=sp1,
                                            tile_position=(0, pos),
                                            skip_group_check=True)

                            # normalization for this head pair
                            den_sb = p2.tile([128, TS], BF, tag="densb",
                                             bufs=2, name="den_sb")
                            nc.vector.tensor_copy(den_sb[:], den[:])
                            bcd = psum2.tile([128, TS], F32, tag="misc",
                                             bufs=1, name="bcd")
                            nc.tensor.matmul(bcd[:], esel_sb[:], den_sb[:],
                                             start=True, stop=True)
                            recf = p2.tile([128, TS], F32, tag="recf",
                                           bufs=2, name="recf")
                            nc.vector.reciprocal_approx_fast(recf[:], bcd[:])
                            recb = p2.tile([128, TS], BF, tag="recb",
                                           bufs=2, name="recb")
                            nc.vector.tensor_copy(recb[:], recf[:])
                            nc.vector.tensor_mul(
                                attn_sb[:, hp, btok], pav[:], recb[:])

                            # interleave previous batch's out-projection
                            if b > 0:
                                emit_oproj(b - 1, hp)
                            # prefetch next batch's V mid-way through
                            if hp == 5 and b + 1 < B:
                                vbs_next = load_vbs(b + 1)

                        if b + 1 < B:
                            vbs_cur = vbs_next

                    for co in range(CT):
                        emit_oproj(B - 1, co)

    nc.compile()
    return nc


def _make_esel():
    E = np.zeros((128, 128), np.float32)
    for p in range(128):
        s = (0 if p < 64 else 64) + (p % 32)
        E[s, p] = 1.0
        E[s + 32, p] = 1.0
    return E.astype(ml_dtypes.bfloat16)


def _prep_inputs(hidden_states, Wq, Wk, Wv, Wo, bo):
    T = hidden_states.shape[1]
    TS = T // N_CORES
    TOK = B * TS
    bf = ml_dtypes.bfloat16
    wqT = np.ascontiguousarray(np.asarray(Wq, np.float32).T).astype(bf)
    wkT = np.ascontiguousarray(np.asarray(Wk, np.float32).T).astype(bf)
    wvT = np.ascontiguousarray(np.asarray(Wv, np.float32).T).astype(bf)
    woT = np.ascontiguousarray(np.asarray(Wo, np.float32).T).astype(bf)
    bo_c = np.asarray(bo, np.float32).reshape(C, 1)
    esel = _make_esel()
    x = np.asarray(hidden_states, np.float32)
    in_maps = []
    for r in range(N_CORES):
        xr = x[:, r * TS:(r + 1) * TS, :].reshape(TOK, C)
        xTr = np.ascontiguousarray(xr.T).astype(bf)
        in_maps.append({
            "xT": xTr, "wqT": wqT, "wkT": wkT, "wvT": wvT, "woT": woT,
            "bo": bo_c, "esel": esel,
        })
    return in_maps


def kernel(hidden_states, Wq, Wk, Wv, Wo, bo):
    global LAST_EXEC_TIME_NS
    _install_ntff_hook()
    Bx, T, Cx = hidden_states.shape
    assert (Bx, Cx) == (B, C)
    TS = T // N_CORES
    if T not in _BUILD_CACHE:
        _BUILD_CACHE[T] = build(T)
    nc = _BUILD_CACHE[T]
    in_maps = _prep_inputs(hidden_states, Wq, Wk, Wv, Wo, bo)
    res = run_bass_kernel_spmd(nc, in_maps, core_ids=list(range(N_CORES)))
    LAST_EXEC_TIME_NS = res.exec_time_ns
    outf = np.empty((B, T, C), np.float32)
    for r in range(N_CORES):
        yT = res.results[r]["out"]          # [C, TOK]
        yr = yT.T.reshape(B, TS, C)
        outf[:, r * TS:(r + 1) * TS, :] = yr
    return outf
